# revision 18
# baseline (speedup 1.0000x reference)
"""CAM (channel-attention) + SE module kernel for TRN2, batch-parallel over 8 cores.

Per sample (C=256, N=9216):
  v = x.reshape(C, N)
  E = v @ v.T                      (energy; fp32r matmuls on PE)
  att = softmax(-E, axis=-1)       (rows; stabilized at row-min of E)
  pooled = mean(x) over N          (ACT accumulate-copies into dead scratch)
  gate = sigmoid(w2 @ relu(w1 @ pooled + b1) + b2)
  out = gamma * gate[:,None] * (att @ v) + x

v5: x is DMA'd straight into SBUF as fp32r (gpsimd cast-DMA rounds in
flight - no staging copy); ident DMA and sample-0 loads lead the ring;
phase2 of sample b interleaves chunk-wise with phase1 of sample b+1 so
stores stream mid-kernel; the softmax chain overlaps the next sample's
transposes (attT emission deferred past the first k2 steps so the
in-order PE queue never head-blocks); phase2 runs in chunk pairs sharing
stationary weights (half the LDWEIGHTS), and the tail alternates the
residual add between DVE and PE-identity-matmul + ACT copy.
"""
import numpy as np
import concourse.bass as bass
import concourse.bacc as bacc
import concourse.tile as tile
import concourse.mybir as mybir
from concourse.bass_utils import run_bass_kernel_spmd
import concourse.bass_utils as _bu

# Re-enable walrus LDWEIGHTS optimization (elides/backgrounds redundant weight
# loads). bass_utils hardcodes --enable-ldw-opt=false; flip it on our compiles.
if not getattr(_bu.run_command, "_ldw_patched", False):
    _orig_run_command = _bu.run_command

    def _run_command_ldw(argv, **kwargs):
        argv = ["--enable-ldw-opt=true" if a == "--enable-ldw-opt=false" else a
                for a in argv]
        return _orig_run_command(argv, **kwargs)

    _run_command_ldw._ldw_patched = True
    _bu.run_command = _run_command_ldw

F32 = mybir.dt.float32
F32R = mybir.dt.float32r

B, C, H, W = 16, 256, 96, 96
N = H * W                 # 9216
NCORES = 8
BL = B // NCORES          # samples per core
NCH = N // 128            # 72 n-chunks for the energy phase
SEG = 1536                # x-load segment columns
NSEG = N // SEG           # 6
R = C // 8                # 32 (SE hidden dim)
NK2 = NCH // 2            # 36 phase-1 double-chunks
NC2 = N // 512            # 18 phase-2 512-col chunks per h
IL0 = 6                   # phase1(1) step at which attT(0) is spliced in
ILK = 8                   # ph2(0) chunks held back past the interleave


def build_nc():
    nc = bacc.Bacc("TRN2", target_bir_lowering=False, debug=False, num_devices=NCORES)

    x_d = nc.dram_tensor("x", [BL, C, N], F32, kind="ExternalInput")
    gamma_d = nc.dram_tensor("gamma", [1], F32, kind="ExternalInput")
    w1_d = nc.dram_tensor("w1", [R, C], F32, kind="ExternalInput")   # pre-scaled by 1/N
    b1_d = nc.dram_tensor("b1", [R], F32, kind="ExternalInput")
    w2_d = nc.dram_tensor("w2", [C, R], F32, kind="ExternalInput")
    b2_d = nc.dram_tensor("b2", [C], F32, kind="ExternalInput")
    ident_d = nc.dram_tensor("ident", [128, 128], F32, kind="ExternalInput")
    out_d = nc.dram_tensor("out", [BL, C, N], F32, kind="ExternalOutput")

    with tile.TileContext(nc) as tc:
        with (
            tc.tile_pool(name="px", bufs=2 * BL) as px,
            tc.tile_pool(name="pxT", bufs=4) as pxT,
            tc.tile_pool(name="pscr", bufs=2) as pscr,
            tc.tile_pool(name="patt", bufs=2) as patt,
            tc.tile_pool(name="pout", bufs=3) as pout,
            tc.tile_pool(name="psmall", bufs=2) as psmall,
            tc.tile_pool(name="psingle", bufs=1) as psingle,
            tc.tile_pool(name="ppsE", bufs=1, space="PSUM") as ppsE,
            tc.tile_pool(name="ppsX", bufs=3, space="PSUM") as ppsX,
            tc.tile_pool(name="ppsO", bufs=4, space="PSUM") as ppsO,
        ):
            # ---------------- ring order: ident, s0 loads, params, s1 loads --
            ident = psingle.tile([128, 128], F32, name="ident")
            nc.gpsimd.dma_start(out=ident[:], in_=ident_d[:])
            identr = psingle.tile([128, 128], F32R, name="identr")
            nc.vector.tensor_copy(out=identr[:], in_=ident[:])

            x_sb = {}
            for b in range(BL):
                x_sb[b] = [
                    px.tile([128, N], F32R, tag="xsb", name=f"x_{b}_{h}")
                    for h in range(2)
                ]

            def emit_loads(b, fine_first=False):
                slices = [slice(SEG * g, SEG * (g + 1)) for g in range(NSEG)]
                if fine_first:
                    slices = [slice(0, SEG // 2), slice(SEG // 2, SEG)] + slices[1:]
                for sl in slices:
                    for h in range(2):
                        nc.gpsimd.dma_start(
                            out=x_sb[b][h][:, sl], in_=x_d[b, 128 * h:128 * (h + 1), sl],
                        )

            emit_loads(0, fine_first=True)

            gamma_sb = psingle.tile([128, 1], F32, name="gamma_sb")
            nc.gpsimd.dma_start(
                out=gamma_sb[:],
                in_=bass.AP(tensor=gamma_d.ap().tensor, offset=0, ap=[[0, 128], [1, 1]]),
            )
            b1_sb = psingle.tile([R, 1], F32, name="b1_sb")
            nc.gpsimd.dma_start(
                out=b1_sb[:],
                in_=bass.AP(tensor=b1_d.ap().tensor, offset=0, ap=[[1, R], [1, 1]]),
            )
            b2_sb = psingle.tile([128, 2], F32, name="b2_sb")
            nc.gpsimd.dma_start(out=b2_sb[:], in_=b2_d[:].rearrange("(h c) -> c h", c=128))
            w1_nat = psingle.tile([R, 2, 128], F32, name="w1_nat")
            nc.gpsimd.dma_start(out=w1_nat[:], in_=w1_d[:].rearrange("r (h c) -> r h c", c=128))
            w2_nat = psingle.tile([128, 2, R], F32, name="w2_nat")
            nc.gpsimd.dma_start(out=w2_nat[:], in_=w2_d[:].rearrange("(h c) r -> c h r", c=128))

            emit_loads(1)

            # preload ACT tables during the otherwise-idle start
            warm = psingle.tile([128, 1], F32, name="warm")
            nc.vector.memset(warm[:], 0.0)
            for fn in (mybir.ActivationFunctionType.Copy,
                       mybir.ActivationFunctionType.Exp,
                       mybir.ActivationFunctionType.Relu,
                       mybir.ActivationFunctionType.Sigmoid):
                nc.scalar.activation(out=warm[:], in_=warm[:], func=fn, scale=1.0)

            # ---------------- per-sample phases ----------------
            psE = {}
            pp = {}
            att = {}
            attT = {}

            def emit_phase1(b, interleave_cb=None):
                """Software-pipelined: transposes of k2 run while mms of k2-1
                execute. xT copies: 2 of 3 on DVE, 1 of 3 on ACT. pooled
                accumulate-copies (ACT, dead scratch output) go at k2=6g+3/4,
                after segment g is consumed (hence loaded). interleave_cb(k2)
                is invoked after every k2."""
                psE[b] = ppsE.tile([128, 512], F32, tag="psE", name=f"psE_{b}")
                pp[b] = psmall.tile([128, 2, NSEG], F32, tag="pp", name=f"pp_{b}")
                pending = []

                def emit_pooled_copy(g, h):
                    sl = slice(SEG * g, SEG * (g + 1))
                    scr = pscr.tile([128, SEG], F32, tag="scr", name=f"scr_{b}_{g}_{h}")
                    nc.scalar.activation(
                        out=scr[:], in_=x_sb[b][h][:, sl],
                        func=mybir.ActivationFunctionType.Copy,
                        accum_out=pp[b][:, h, g:g + 1],
                    )

                def emit_mms(k2, xT):
                    for sub in range(2):
                        for h in range(2):
                            nc.tensor.matmul(
                                psE[b][:, 256 * h:256 * (h + 1)],
                                xT[:, 256 * sub + 128 * h:256 * sub + 128 * (h + 1)],
                                xT[:, 256 * sub:256 * (sub + 1)],
                                start=(k2 == 0 and sub == 0 and h == 0),
                                stop=(k2 == NK2 - 1 and sub == 1 and h == 1),
                            )

                for k2 in range(NK2):
                    xT_ps = ppsX.tile([128, 512], F32R, tag="psx", name=f"xTps_{b}_{k2}")
                    for sub in range(2):
                        k = 2 * k2 + sub
                        for h in range(2):
                            nc.tensor.transpose(
                                xT_ps[:, 256 * sub + 128 * h:256 * sub + 128 * (h + 1)],
                                x_sb[b][h][:, 128 * k:128 * (k + 1)],
                                identr[:],
                            )
                    xT = pxT.tile([128, 512], F32R, tag="xT", name=f"xT_{b}_{k2}")
                    if k2 % 3 == 2:
                        nc.scalar.copy(out=xT[:], in_=xT_ps[:])
                    else:
                        nc.vector.tensor_copy(out=xT[:], in_=xT_ps[:])
                    pending.append((k2, xT))
                    if len(pending) > 2:
                        emit_mms(*pending.pop(0))
                    if k2 % 6 == 1:
                        emit_pooled_copy(k2 // 6, 0)
                    elif k2 % 6 == 2:
                        emit_pooled_copy(k2 // 6, 1)
                    if interleave_cb is not None:
                        interleave_cb(k2)
                for args in pending:
                    emit_mms(*args)

            def emit_wprep():
                # w1T[c, h, r] = w1[r, h*128+c]
                w1T_ps = ppsX.tile([128, 2, R], F32, tag="psx", name="w1T_ps")
                for h in range(2):
                    nc.tensor.transpose(w1T_ps[:, h, :], w1_nat[:, h, :], ident[0:R, 0:R])
                w1T = psingle.tile([128, 2, R], F32, name="w1T")
                nc.vector.tensor_copy(out=w1T[:], in_=w1T_ps[:])
                # w2T[r, h*128+c] = w2[h*128+c, r]
                w2T = psingle.tile([R, 2, 128], F32, name="w2T")
                for h in range(2):
                    w2T_ps = ppsX.tile([R, 128], F32, tag="psx", name=f"w2T_ps_{h}")
                    nc.tensor.transpose(w2T_ps[:], w2_nat[:, h, :], ident[:])
                    nc.vector.tensor_copy(out=w2T[:, h, :], in_=w2T_ps[:])
                return w1T, w2T

            def emit_gate_softmax(b, w1T, w2T):
                """SE gate + softmax rows (everything except the attT
                transposes, which are deferred to overlap the next phase)."""
                pooled = psmall.tile([128, 2], F32, tag="pooled", name=f"pooled_{b}")
                for h in range(2):
                    nc.vector.reduce_sum(
                        out=pooled[:, h:h + 1], in_=pp[b][:, h, :], axis=mybir.AxisListType.X,
                    )
                hid_ps = ppsX.tile([R, 1], F32, tag="psx", name=f"hid_ps_{b}")
                for h in range(2):
                    nc.tensor.matmul(
                        hid_ps[:], w1T[:, h, :], pooled[:, h:h + 1],
                        start=(h == 0), stop=(h == 1),
                    )
                hid = psmall.tile([R, 1], F32, tag="hid", name=f"hid_{b}")
                nc.scalar.activation(
                    out=hid[:], in_=hid_ps[:],
                    func=mybir.ActivationFunctionType.Relu, bias=b1_sb[:], scale=1.0,
                )
                gg = psmall.tile([128, 2], F32, tag="gg", name=f"gg_{b}")
                for h in range(2):
                    gate_ps = ppsX.tile([128, 1], F32, tag="psx", name=f"gate_ps_{b}_{h}")
                    nc.tensor.matmul(gate_ps[:], w2T[:, h, :], hid[:])
                    nc.scalar.activation(
                        out=gg[:, h:h + 1], in_=gate_ps[:],
                        func=mybir.ActivationFunctionType.Sigmoid,
                        bias=b2_sb[:, h:h + 1], scale=1.0,
                    )
                nc.vector.tensor_scalar_mul(out=gg[:], in0=gg[:], scalar1=gamma_sb[:])

                # softmax rows + fold in gamma*gate
                att[b] = []
                for h in range(2):
                    pE = psE[b][:, 256 * h:256 * (h + 1)]
                    mn = psmall.tile([128, 1], F32, tag="mn", name=f"mn_{b}_{h}")
                    nc.vector.tensor_reduce(
                        out=mn[:], in_=pE,
                        axis=mybir.AxisListType.X, op=mybir.AluOpType.min,
                    )
                    s = psmall.tile([128, 1], F32, tag="s", name=f"s_{b}_{h}")
                    at = patt.tile([128, 256], F32, tag=f"att{h}", bufs=1, name=f"att_{b}_{h}")
                    nc.scalar.activation(
                        out=at[:], in_=pE,
                        func=mybir.ActivationFunctionType.Exp,
                        bias=mn[:], scale=-1.0, accum_out=s[:],
                    )
                    rs = psmall.tile([128, 1], F32, tag="rs", name=f"rs_{b}_{h}")
                    nc.vector.reciprocal(out=rs[:], in_=s[:])
                    nc.vector.tensor_mul(out=rs[:], in0=rs[:], in1=gg[:, h:h + 1])
                    nc.vector.tensor_scalar_mul(out=at[:], in0=at[:], scalar1=rs[:])
                    att[b].append(at)

            def emit_attT(b):
                # transpose attention (f32 PE transpose, round on ACT copy)
                attT[b] = patt.tile([128, 2, 256], F32R, tag="attT", name=f"attT_{b}")
                for j in range(2):
                    attT_ps = ppsX.tile([128, 256], F32, tag="psx", name=f"attTps_{b}_{j}")
                    for h in range(2):
                        nc.tensor.transpose(
                            attT_ps[:, 128 * h:128 * (h + 1)],
                            att[b][h][:, 128 * j:128 * (j + 1)],
                            ident[:],
                        )
                    nc.scalar.copy(out=attT[b][:, j, :], in_=attT_ps[:])

            def make_phase2_pairs(b, ident_split):
                """One emitter per PAIR of [128, 512] output chunks. The two
                chunks share stationary weights (mm order a-j0, b-j0, a-j1,
                b-j1 halves LDWEIGHTS). Residual: chunk a via DVE add; chunk b
                via PE identity-matmul + ACT copy when ident_split, else DVE.
                SP store per filled [128, 2048] o_sb."""
                pairs = []
                state = {"o_sb": None, "col0": 0}

                def make(h, p2):
                    def emit():
                        c2a, c2b = 2 * p2, 2 * p2 + 1
                        n0a, n0b = 512 * c2a, 512 * c2b
                        if c2a % 4 == 0:
                            state["o_sb"] = pout.tile([128, 2048], F32, tag="osb",
                                                      name=f"o_{b}_{h}_{p2}")
                            state["col0"] = n0a
                        o_sb = state["o_sb"]
                        pso_a = ppsO.tile([128, 512], F32, tag="ps_o", name=f"psoa_{b}_{h}_{p2}")
                        pso_b = ppsO.tile([128, 512], F32, tag="ps_o", name=f"psob_{b}_{h}_{p2}")
                        for j in range(2):
                            for pso, n0 in ((pso_a, n0a), (pso_b, n0b)):
                                nc.tensor.matmul(
                                    pso[:],
                                    attT[b][:, j, 128 * h:128 * (h + 1)],
                                    x_sb[b][j][:, n0:n0 + 512],
                                    start=(j == 0),
                                    stop=(j == 1 and not (ident_split and pso is pso_b)),
                                )
                        if ident_split:
                            nc.tensor.matmul(
                                pso_b[:], identr[:],
                                x_sb[b][h][:, n0b:n0b + 512],
                                start=False, stop=True,
                            )
                        offa = n0a - state["col0"]
                        nc.vector.tensor_add(
                            out=o_sb[:, offa:offa + 512],
                            in0=pso_a[:],
                            in1=x_sb[b][h][:, n0a:n0a + 512],
                        )
                        offb = offa + 512
                        if ident_split:
                            nc.scalar.copy(out=o_sb[:, offb:offb + 512], in_=pso_b[:])
                        else:
                            nc.vector.tensor_add(
                                out=o_sb[:, offb:offb + 512],
                                in0=pso_b[:],
                                in1=x_sb[b][h][:, n0b:n0b + 512],
                            )
                        if ident_split or c2b % 4 == 3 or c2b == NC2 - 1:
                            cw = offb + 512
                            nc.sync.dma_start(
                                out=out_d[b, 128 * h:128 * (h + 1),
                                          state["col0"] + (cw - 1024):state["col0"] + cw],
                                in_=o_sb[:, cw - 1024:cw],
                            ) if ident_split and cw > 1024 else nc.sync.dma_start(
                                out=out_d[b, 128 * h:128 * (h + 1),
                                          state["col0"]:state["col0"] + cw],
                                in_=o_sb[:, 0:cw],
                            )
                    return emit

                for h in range(2):
                    for p2 in range(NC2 // 2):
                        pairs.append(make(h, p2))
                return pairs

            # ---------------- schedule ----------------
            emit_phase1(0)
            w1T, w2T = emit_wprep()
            emit_gate_softmax(0, w1T, w2T)

            ph2_0 = make_phase2_pairs(0, ident_split=False)
            n_pairs = len(ph2_0)          # 18
            hold = ILK // 2               # pairs held back for the epi(1) gap
            consumed = [0]

            def splice(k2):
                if k2 == IL0:
                    emit_attT(0)
                if k2 > IL0:
                    target = round((k2 - IL0) * (n_pairs - hold) / (NK2 - 1 - IL0))
                    while consumed[0] < target:
                        ph2_0[consumed[0]]()
                        consumed[0] += 1

            emit_phase1(1, interleave_cb=splice)
            rest = ph2_0[consumed[0]:]
            for emit in rest[:len(rest) // 2]:
                emit()
            emit_gate_softmax(1, w1T, w2T)
            for emit in rest[len(rest) // 2:]:
                emit()
            emit_attT(1)
            for emit in make_phase2_pairs(1, ident_split=False):
                emit()

    nc.finalize()
    return nc


_CACHE = {}


def get_nc():
    if "nc" not in _CACHE:
        _CACHE["nc"] = build_nc()
    return _CACHE["nc"]


def kernel_with_result(x, gamma, w1, b1, w2, b2, trace=False, **_ignored):
    x = np.asarray(x, dtype=np.float32)
    nc = get_nc()
    params = {
        "gamma": np.asarray(gamma, np.float32).reshape(1),
        "w1": np.asarray(w1, np.float32) * np.float32(1.0 / N),
        "b1": np.asarray(b1, np.float32),
        "w2": np.asarray(w2, np.float32),
        "b2": np.asarray(b2, np.float32),
        "ident": np.eye(128, dtype=np.float32),
    }
    xr = x.reshape(B, C, N)
    in_maps = [dict(params, x=xr[i * BL:(i + 1) * BL]) for i in range(NCORES)]
    res = run_bass_kernel_spmd(nc, in_maps, core_ids=list(range(NCORES)), trace=trace)
    out = np.concatenate([res.results[i]["out"] for i in range(NCORES)], axis=0)
    return out.reshape(B, C, H, W), res


def kernel(x, gamma, w1, b1, w2, b2, **_ignored):
    out, _res = kernel_with_result(x, gamma, w1, b1, w2, b2, trace=False)
    return out


# revision 19
# speedup vs baseline: 1.0205x; 1.0205x over previous
"""CAM (channel-attention) + SE module kernel for TRN2, batch-parallel over 8 cores.

Per sample (C=256, N=9216):
  v = x.reshape(C, N)
  E = v @ v.T                      (energy; fp32r matmuls on PE)
  att = softmax(-E, axis=-1)       (rows; stabilized at row-min of E)
  pooled = mean(x) over N          (ACT accumulate-copies into dead scratch)
  gate = sigmoid(w2 @ relu(w1 @ pooled + b1) + b2)
  out = gamma * gate[:,None] * (att @ v) + x

v5: x is DMA'd straight into SBUF as fp32r (gpsimd cast-DMA rounds in
flight - no staging copy); ident DMA and sample-0 loads lead the ring;
phase2 of sample b interleaves chunk-wise with phase1 of sample b+1 so
stores stream mid-kernel; the softmax chain overlaps the next sample's
transposes (attT emission deferred past the first k2 steps so the
in-order PE queue never head-blocks); phase2 runs in chunk pairs sharing
stationary weights (half the LDWEIGHTS), and the tail alternates the
residual add between DVE and PE-identity-matmul + ACT copy.
"""
import numpy as np
import concourse.bass as bass
import concourse.bacc as bacc
import concourse.tile as tile
import concourse.mybir as mybir
from concourse.bass_utils import run_bass_kernel_spmd
import concourse.bass_utils as _bu

# Re-enable walrus LDWEIGHTS optimization (elides/backgrounds redundant weight
# loads). bass_utils hardcodes --enable-ldw-opt=false; flip it on our compiles.
if not getattr(_bu.run_command, "_ldw_patched", False):
    _orig_run_command = _bu.run_command

    def _run_command_ldw(argv, **kwargs):
        argv = ["--enable-ldw-opt=true" if a == "--enable-ldw-opt=false" else a
                for a in argv]
        return _orig_run_command(argv, **kwargs)

    _run_command_ldw._ldw_patched = True
    _bu.run_command = _run_command_ldw

F32 = mybir.dt.float32
F32R = mybir.dt.float32r

B, C, H, W = 16, 256, 96, 96
N = H * W                 # 9216
NCORES = 8
BL = B // NCORES          # samples per core
NCH = N // 128            # 72 n-chunks for the energy phase
SEG = 1536                # x-load segment columns
NSEG = N // SEG           # 6
R = C // 8                # 32 (SE hidden dim)
NK2 = NCH // 2            # 36 phase-1 double-chunks
NC2 = N // 512            # 18 phase-2 512-col chunks per h
IL0 = 6                   # phase1(1) step at which attT(0) is spliced in
ILK = 12                  # ph2(0) chunks held back past the interleave


def build_nc():
    nc = bacc.Bacc("TRN2", target_bir_lowering=False, debug=False, num_devices=NCORES)

    x_d = nc.dram_tensor("x", [BL, C, N], F32, kind="ExternalInput")
    gamma_d = nc.dram_tensor("gamma", [1], F32, kind="ExternalInput")
    w1_d = nc.dram_tensor("w1", [R, C], F32, kind="ExternalInput")   # pre-scaled by 1/N
    b1_d = nc.dram_tensor("b1", [R], F32, kind="ExternalInput")
    w2_d = nc.dram_tensor("w2", [C, R], F32, kind="ExternalInput")
    b2_d = nc.dram_tensor("b2", [C], F32, kind="ExternalInput")
    ident_d = nc.dram_tensor("ident", [128, 128], F32, kind="ExternalInput")
    out_d = nc.dram_tensor("out", [BL, C, N], F32, kind="ExternalOutput")

    with tile.TileContext(nc) as tc:
        with (
            tc.tile_pool(name="px", bufs=2 * BL) as px,
            tc.tile_pool(name="pxT", bufs=4) as pxT,
            tc.tile_pool(name="pscr", bufs=2) as pscr,
            tc.tile_pool(name="patt", bufs=2) as patt,
            tc.tile_pool(name="pout", bufs=3) as pout,
            tc.tile_pool(name="psmall", bufs=2) as psmall,
            tc.tile_pool(name="psingle", bufs=1) as psingle,
            tc.tile_pool(name="ppsE", bufs=1, space="PSUM") as ppsE,
            tc.tile_pool(name="ppsX", bufs=3, space="PSUM") as ppsX,
            tc.tile_pool(name="ppsO", bufs=4, space="PSUM") as ppsO,
        ):
            # ---------------- ring order: ident, s0 loads, params, s1 loads --
            ident = psingle.tile([128, 128], F32, name="ident")
            nc.gpsimd.dma_start(out=ident[:], in_=ident_d[:])
            identr = psingle.tile([128, 128], F32R, name="identr")
            nc.vector.tensor_copy(out=identr[:], in_=ident[:])

            x_sb = {}
            for b in range(BL):
                x_sb[b] = [
                    px.tile([128, N], F32R, tag="xsb", name=f"x_{b}_{h}")
                    for h in range(2)
                ]

            def emit_loads(b, fine_first=False):
                slices = [slice(SEG * g, SEG * (g + 1)) for g in range(NSEG)]
                if fine_first:
                    slices = [slice(0, SEG // 2), slice(SEG // 2, SEG)] + slices[1:]
                for sl in slices:
                    for h in range(2):
                        nc.gpsimd.dma_start(
                            out=x_sb[b][h][:, sl], in_=x_d[b, 128 * h:128 * (h + 1), sl],
                        )

            emit_loads(0, fine_first=True)

            gamma_sb = psingle.tile([128, 1], F32, name="gamma_sb")
            nc.gpsimd.dma_start(
                out=gamma_sb[:],
                in_=bass.AP(tensor=gamma_d.ap().tensor, offset=0, ap=[[0, 128], [1, 1]]),
            )
            b1_sb = psingle.tile([R, 1], F32, name="b1_sb")
            nc.gpsimd.dma_start(
                out=b1_sb[:],
                in_=bass.AP(tensor=b1_d.ap().tensor, offset=0, ap=[[1, R], [1, 1]]),
            )
            b2_sb = psingle.tile([128, 2], F32, name="b2_sb")
            nc.gpsimd.dma_start(out=b2_sb[:], in_=b2_d[:].rearrange("(h c) -> c h", c=128))
            w1_nat = psingle.tile([R, 2, 128], F32, name="w1_nat")
            nc.gpsimd.dma_start(out=w1_nat[:], in_=w1_d[:].rearrange("r (h c) -> r h c", c=128))
            w2_nat = psingle.tile([128, 2, R], F32, name="w2_nat")
            nc.gpsimd.dma_start(out=w2_nat[:], in_=w2_d[:].rearrange("(h c) r -> c h r", c=128))

            emit_loads(1)

            # preload ACT tables during the otherwise-idle start
            warm = psingle.tile([128, 1], F32, name="warm")
            nc.vector.memset(warm[:], 0.0)
            for fn in (mybir.ActivationFunctionType.Copy,
                       mybir.ActivationFunctionType.Exp,
                       mybir.ActivationFunctionType.Relu,
                       mybir.ActivationFunctionType.Sigmoid):
                nc.scalar.activation(out=warm[:], in_=warm[:], func=fn, scale=1.0)

            # ---------------- per-sample phases ----------------
            psE = {}
            pp = {}
            att = {}
            attT = {}

            def emit_phase1(b, interleave_cb=None):
                """Software-pipelined: transposes of k2 run while mms of k2-1
                execute. xT copies: 2 of 3 on DVE, 1 of 3 on ACT. pooled
                accumulate-copies (ACT, dead scratch output) go at k2=6g+3/4,
                after segment g is consumed (hence loaded). interleave_cb(k2)
                is invoked after every k2."""
                psE[b] = ppsE.tile([128, 512], F32, tag="psE", name=f"psE_{b}")
                pp[b] = psmall.tile([128, 2, NSEG], F32, tag="pp", name=f"pp_{b}")
                pending = []

                def emit_pooled_copy(g, h):
                    sl = slice(SEG * g, SEG * (g + 1))
                    scr = pscr.tile([128, SEG], F32, tag="scr", name=f"scr_{b}_{g}_{h}")
                    nc.scalar.activation(
                        out=scr[:], in_=x_sb[b][h][:, sl],
                        func=mybir.ActivationFunctionType.Copy,
                        accum_out=pp[b][:, h, g:g + 1],
                    )

                def emit_mms(k2, xT):
                    for sub in range(2):
                        for h in range(2):
                            nc.tensor.matmul(
                                psE[b][:, 256 * h:256 * (h + 1)],
                                xT[:, 256 * sub + 128 * h:256 * sub + 128 * (h + 1)],
                                xT[:, 256 * sub:256 * (sub + 1)],
                                start=(k2 == 0 and sub == 0 and h == 0),
                                stop=(k2 == NK2 - 1 and sub == 1 and h == 1),
                            )

                for k2 in range(NK2):
                    xT_ps = ppsX.tile([128, 512], F32R, tag="psx", name=f"xTps_{b}_{k2}")
                    for sub in range(2):
                        k = 2 * k2 + sub
                        for h in range(2):
                            nc.tensor.transpose(
                                xT_ps[:, 256 * sub + 128 * h:256 * sub + 128 * (h + 1)],
                                x_sb[b][h][:, 128 * k:128 * (k + 1)],
                                identr[:],
                            )
                    xT = pxT.tile([128, 512], F32R, tag="xT", name=f"xT_{b}_{k2}")
                    if k2 % 3 == 2:
                        nc.scalar.copy(out=xT[:], in_=xT_ps[:])
                    else:
                        nc.vector.tensor_copy(out=xT[:], in_=xT_ps[:])
                    pending.append((k2, xT))
                    if len(pending) > 2:
                        emit_mms(*pending.pop(0))
                    if k2 % 6 == 1:
                        emit_pooled_copy(k2 // 6, 0)
                    elif k2 % 6 == 2:
                        emit_pooled_copy(k2 // 6, 1)
                    if interleave_cb is not None:
                        interleave_cb(k2)
                for args in pending:
                    emit_mms(*args)

            def emit_wprep():
                # w1T[c, h, r] = w1[r, h*128+c]
                w1T_ps = ppsX.tile([128, 2, R], F32, tag="psx", name="w1T_ps")
                for h in range(2):
                    nc.tensor.transpose(w1T_ps[:, h, :], w1_nat[:, h, :], ident[0:R, 0:R])
                w1T = psingle.tile([128, 2, R], F32, name="w1T")
                nc.vector.tensor_copy(out=w1T[:], in_=w1T_ps[:])
                # w2T[r, h*128+c] = w2[h*128+c, r]
                w2T = psingle.tile([R, 2, 128], F32, name="w2T")
                for h in range(2):
                    w2T_ps = ppsX.tile([R, 128], F32, tag="psx", name=f"w2T_ps_{h}")
                    nc.tensor.transpose(w2T_ps[:], w2_nat[:, h, :], ident[:])
                    nc.vector.tensor_copy(out=w2T[:, h, :], in_=w2T_ps[:])
                return w1T, w2T

            def emit_gate_softmax(b, w1T, w2T):
                """SE gate + softmax rows (everything except the attT
                transposes, which are deferred to overlap the next phase)."""
                pooled = psmall.tile([128, 2], F32, tag="pooled", name=f"pooled_{b}")
                for h in range(2):
                    nc.vector.reduce_sum(
                        out=pooled[:, h:h + 1], in_=pp[b][:, h, :], axis=mybir.AxisListType.X,
                    )
                hid_ps = ppsX.tile([R, 1], F32, tag="psx", name=f"hid_ps_{b}")
                for h in range(2):
                    nc.tensor.matmul(
                        hid_ps[:], w1T[:, h, :], pooled[:, h:h + 1],
                        start=(h == 0), stop=(h == 1),
                    )
                hid = psmall.tile([R, 1], F32, tag="hid", name=f"hid_{b}")
                nc.scalar.activation(
                    out=hid[:], in_=hid_ps[:],
                    func=mybir.ActivationFunctionType.Relu, bias=b1_sb[:], scale=1.0,
                )
                gg = psmall.tile([128, 2], F32, tag="gg", name=f"gg_{b}")
                for h in range(2):
                    gate_ps = ppsX.tile([128, 1], F32, tag="psx", name=f"gate_ps_{b}_{h}")
                    nc.tensor.matmul(gate_ps[:], w2T[:, h, :], hid[:])
                    nc.scalar.activation(
                        out=gg[:, h:h + 1], in_=gate_ps[:],
                        func=mybir.ActivationFunctionType.Sigmoid,
                        bias=b2_sb[:, h:h + 1], scale=1.0,
                    )
                nc.vector.tensor_scalar_mul(out=gg[:], in0=gg[:], scalar1=gamma_sb[:])

                # softmax rows + fold in gamma*gate
                att[b] = []
                for h in range(2):
                    pE = psE[b][:, 256 * h:256 * (h + 1)]
                    mn = psmall.tile([128, 1], F32, tag="mn", name=f"mn_{b}_{h}")
                    nc.vector.tensor_reduce(
                        out=mn[:], in_=pE,
                        axis=mybir.AxisListType.X, op=mybir.AluOpType.min,
                    )
                    s = psmall.tile([128, 1], F32, tag="s", name=f"s_{b}_{h}")
                    at = patt.tile([128, 256], F32, tag=f"att{h}", bufs=1, name=f"att_{b}_{h}")
                    nc.scalar.activation(
                        out=at[:], in_=pE,
                        func=mybir.ActivationFunctionType.Exp,
                        bias=mn[:], scale=-1.0, accum_out=s[:],
                    )
                    rs = psmall.tile([128, 1], F32, tag="rs", name=f"rs_{b}_{h}")
                    nc.vector.reciprocal(out=rs[:], in_=s[:])
                    nc.vector.tensor_mul(out=rs[:], in0=rs[:], in1=gg[:, h:h + 1])
                    nc.vector.tensor_scalar_mul(out=at[:], in0=at[:], scalar1=rs[:])
                    att[b].append(at)

            def emit_attT(b):
                # transpose attention (f32 PE transpose, round on ACT copy)
                attT[b] = patt.tile([128, 2, 256], F32R, tag="attT", name=f"attT_{b}")
                for j in range(2):
                    attT_ps = ppsX.tile([128, 256], F32, tag="psx", name=f"attTps_{b}_{j}")
                    for h in range(2):
                        nc.tensor.transpose(
                            attT_ps[:, 128 * h:128 * (h + 1)],
                            att[b][h][:, 128 * j:128 * (j + 1)],
                            ident[:],
                        )
                    nc.scalar.copy(out=attT[b][:, j, :], in_=attT_ps[:])

            def make_phase2_pairs(b, ident_split):
                """One emitter per PAIR of [128, 512] output chunks. The two
                chunks share stationary weights (mm order a-j0, b-j0, a-j1,
                b-j1 halves LDWEIGHTS). Residual: chunk a via DVE add; chunk b
                via PE identity-matmul + ACT copy when ident_split, else DVE.
                SP store per filled [128, 2048] o_sb."""
                pairs = []
                state = {"o_sb": None, "col0": 0}

                def make(h, p2):
                    def emit():
                        c2a, c2b = 2 * p2, 2 * p2 + 1
                        n0a, n0b = 512 * c2a, 512 * c2b
                        if c2a % 4 == 0:
                            state["o_sb"] = pout.tile([128, 2048], F32, tag="osb",
                                                      name=f"o_{b}_{h}_{p2}")
                            state["col0"] = n0a
                        o_sb = state["o_sb"]
                        pso_a = ppsO.tile([128, 512], F32, tag="ps_o", name=f"psoa_{b}_{h}_{p2}")
                        pso_b = ppsO.tile([128, 512], F32, tag="ps_o", name=f"psob_{b}_{h}_{p2}")
                        for j in range(2):
                            for pso, n0 in ((pso_a, n0a), (pso_b, n0b)):
                                nc.tensor.matmul(
                                    pso[:],
                                    attT[b][:, j, 128 * h:128 * (h + 1)],
                                    x_sb[b][j][:, n0:n0 + 512],
                                    start=(j == 0),
                                    stop=(j == 1 and not (ident_split and pso is pso_b)),
                                )
                        if ident_split:
                            nc.tensor.matmul(
                                pso_b[:], identr[:],
                                x_sb[b][h][:, n0b:n0b + 512],
                                start=False, stop=True,
                            )
                        offa = n0a - state["col0"]
                        nc.vector.tensor_add(
                            out=o_sb[:, offa:offa + 512],
                            in0=pso_a[:],
                            in1=x_sb[b][h][:, n0a:n0a + 512],
                        )
                        offb = offa + 512
                        if ident_split:
                            nc.scalar.copy(out=o_sb[:, offb:offb + 512], in_=pso_b[:])
                        else:
                            nc.vector.tensor_add(
                                out=o_sb[:, offb:offb + 512],
                                in0=pso_b[:],
                                in1=x_sb[b][h][:, n0b:n0b + 512],
                            )
                        if ident_split or c2b % 4 == 3 or c2b == NC2 - 1:
                            cw = offb + 512
                            nc.sync.dma_start(
                                out=out_d[b, 128 * h:128 * (h + 1),
                                          state["col0"] + (cw - 1024):state["col0"] + cw],
                                in_=o_sb[:, cw - 1024:cw],
                            ) if ident_split and cw > 1024 else nc.sync.dma_start(
                                out=out_d[b, 128 * h:128 * (h + 1),
                                          state["col0"]:state["col0"] + cw],
                                in_=o_sb[:, 0:cw],
                            )
                    return emit

                for h in range(2):
                    for p2 in range(NC2 // 2):
                        pairs.append(make(h, p2))
                return pairs

            # ---------------- schedule ----------------
            emit_phase1(0)
            w1T, w2T = emit_wprep()
            emit_gate_softmax(0, w1T, w2T)

            ph2_0 = make_phase2_pairs(0, ident_split=False)
            n_pairs = len(ph2_0)          # 18
            hold = ILK // 2               # pairs held back for the epi(1) gap
            consumed = [0]

            def splice(k2):
                if k2 == IL0:
                    emit_attT(0)
                if k2 > IL0:
                    target = round((k2 - IL0) * (n_pairs - hold) / (NK2 - 1 - IL0))
                    while consumed[0] < target:
                        ph2_0[consumed[0]]()
                        consumed[0] += 1

            emit_phase1(1, interleave_cb=splice)
            rest = ph2_0[consumed[0]:]
            for emit in rest[:len(rest) // 2]:
                emit()
            emit_gate_softmax(1, w1T, w2T)
            for emit in rest[len(rest) // 2:]:
                emit()
            emit_attT(1)
            for emit in make_phase2_pairs(1, ident_split=True):
                emit()

    nc.finalize()
    return nc


_CACHE = {}


def get_nc():
    if "nc" not in _CACHE:
        _CACHE["nc"] = build_nc()
    return _CACHE["nc"]


def kernel_with_result(x, gamma, w1, b1, w2, b2, trace=False, **_ignored):
    x = np.asarray(x, dtype=np.float32)
    nc = get_nc()
    params = {
        "gamma": np.asarray(gamma, np.float32).reshape(1),
        "w1": np.asarray(w1, np.float32) * np.float32(1.0 / N),
        "b1": np.asarray(b1, np.float32),
        "w2": np.asarray(w2, np.float32),
        "b2": np.asarray(b2, np.float32),
        "ident": np.eye(128, dtype=np.float32),
    }
    xr = x.reshape(B, C, N)
    in_maps = [dict(params, x=xr[i * BL:(i + 1) * BL]) for i in range(NCORES)]
    res = run_bass_kernel_spmd(nc, in_maps, core_ids=list(range(NCORES)), trace=trace)
    out = np.concatenate([res.results[i]["out"] for i in range(NCORES)], axis=0)
    return out.reshape(B, C, H, W), res


def kernel(x, gamma, w1, b1, w2, b2, **_ignored):
    out, _res = kernel_with_result(x, gamma, w1, b1, w2, b2, trace=False)
    return out
